# revision 10
# baseline (speedup 1.0000x reference)
"""Multi-head attention (B=4, G=2048, E=768, H=4) on 8 TRN2 NeuronCores.

Sharding v2 (tensor-parallel over heads): core c = (batch b = c//2,
head-group hg = c%2). Each core computes Q/K/V for ITS 2 heads over all
2048 tokens (no duplicated K/V work), attention for its heads over all
queries, and a PARTIAL output projection (contraction over its heads'
384 d-rows). A pair-wise HBM ReduceScatter(add) then writes each core's
1024-query half of the final output directly into y.

Device dataflow (all heavy matmuls in f16, fp32 PSUM accumulation):
  xT resident in SBUF (contraction dim on partitions).
  V phase:   Vext[hl][tt] (128 tok, 192+1) = (x @ Wv_hg + bv) per
             local-head/token-tile (one 384-wide matmul per chunk).
  QK phase:  KT/QT tiles (128 c, 2048 tok) = (x @ Wqk_hg + b)^T,
             c = (local head, d) in 3 kt + 3 qt tiles of 128.
  Attention: per (512-query block qb, local head hl): ET pairs = two
             key-tiles per PSUM tile -> ONE exp activation per pair ->
             avT(d,q) accumulated over 16 key tiles; row sums via ones
             column; normalize via reciprocal + broadcast matmul.
  Proj:      partial[q,e] = avs^T @ Wp_hg (+ bp on hg0 only), DMA per
             128-query tile into y (2048,768) f16, global query order.
  Reduce:    the pair-wise partial-sum add happens on the HOST during
             unshard (y_pair0 + y_pair1, computed in f32) -- keeps the
             device critical path free of collectives.
"""
import sys

sys.path.insert(0, "/opt/trn_rl_repo")
sys.path.insert(0, "/root/.axon_site")

from contextlib import ExitStack

import numpy as np

import concourse.bass as bass
import concourse.tile as tile
from concourse import bacc, mybir
from concourse.bass_utils import run_bass_kernel_spmd

N_CORES = 8
B, G, E, H = 4, 2048, 768, 4
D = E // H            # 192
HL = 2                # local heads per core
DL = HL * D           # 384 local d-rows
HALF = G // 2         # 1024 output rows per core
KCH = E // 128        # 6 contraction chunks
SCALE = 1.0 / float(np.sqrt(E))
RG = [[0, 1], [2, 3], [4, 5], [6, 7]]

f32 = mybir.dt.float32
f16 = mybir.dt.float16


def _c_chunks(hl):
    """Split local head hl's c-range [hl*192,(hl+1)*192) on 128 boundaries."""
    out = []
    c, c1 = hl * D, (hl + 1) * D
    while c < c1:
        ti, off = divmod(c, 128)
        ln = min(128 - off, c1 - c)
        out.append((ti, off, ln))
        c += ln
    return out


def _emit(nc, t):
    with ExitStack() as top:
        tc = top.enter_context(tile.TileContext(nc))
        const = top.enter_context(tc.tile_pool(name="const", bufs=1))
        kqt_p = top.enter_context(tc.tile_pool(name="kqt", bufs=1))
        v_p = top.enter_context(tc.tile_pool(name="vext", bufs=1))

        # consts go FIRST on the sync queue: tiny transfers queued after the
        # bulk xt/weight DMAs otherwise complete ~16us in and gate the first
        # (bias-broadcast) matmuls. f16 ones/biases keep every matmul off
        # the 4-cycles-per-row fp32 path.
        ones1 = const.tile([1, 128], f16, tag="ones1")
        nc.vector.memset(ones1[:], 1.0)
        onesK = const.tile([128, 1], f16, tag="onesK")
        nc.sync.dma_start(onesK[:], t["onesk"][:])
        bqk_sb = const.tile([128, 6], f32, tag="bqk")
        nc.sync.dma_start(bqk_sb[:], t["bqk"][:])
        bv_sb = const.tile([1, DL], f16, tag="bv")
        nc.sync.dma_start(bv_sb[:], t["bv"][:])
        bp_sb = const.tile([1, E], f16, tag="bp")
        nc.sync.dma_start(bp_sb[:], t["bp"][:])
        bv_bc = const.tile([128, DL], f32, tag="bv_bc")
        bp_bc = const.tile([128, E], f32, tag="bp_bc")

        kt_sb = [kqt_p.tile([128, G], f16, tag=f"kt{i}", name=f"kt{i}")
                 for i in range(3)]
        qt_sb = [kqt_p.tile([128, G], f16, tag=f"qt{i}", name=f"qt{i}")
                 for i in range(3)]
        vext = [[v_p.tile([128, D + 1], f16, tag=f"v{hl}_{tt}",
                          name=f"v{hl}_{tt}")
                 for tt in range(16)] for hl in range(HL)]

        with tc.tile_pool(name="xt_pool", bufs=1) as xt_p, \
             tc.tile_pool(name="wqk_pool", bufs=1) as wqkp, \
             tc.tile_pool(name="vqkps", bufs=1, space="PSUM") as vps:
            xt = xt_p.tile([128, KCH * G], f16, tag="xt")
            wqk_sb = wqkp.tile([128, 6 * 768], f16, tag="wqk")
            qkps = vps

            # ---- V phase (one PSUM pool spans V+QK: no inter-phase drain) -
            with tc.tile_pool(name="wv_pool", bufs=1) as wvp:
                # wv/wqk as single DMAs (descriptor gen ~650ns apiece on the
                # sync queue); xt stays chunked so chunk 0 lands early and
                # the V phase starts as soon as possible
                wv_sb = wvp.tile([128, KCH * DL], f16, tag="wv")
                # the DMA fabric ramps slowly over the first ~10us, so land
                # exactly the first V burst's operands (xt chunk0 cols 0:512
                # + wv chunk0, ~0.2MB) before the bulk transfers
                nc.sync.dma_start(xt[:, 0:512], t["xt"][:, 0:512])
                nc.sync.dma_start(wv_sb[:, 0:DL], t["wv"][:, 0:DL])
                nc.sync.dma_start(xt[:, 512:G], t["xt"][:, 512:G])
                nc.sync.dma_start(wv_sb[:, DL:], t["wv"][:, DL:])
                # spread the bulk xt chunks across three engines' DMA queues
                # so the transfers parallelize during the slow early ramp
                qeng = [nc.gpsimd, nc.scalar, nc.sync]
                for k in range(1, KCH):
                    qeng[k % 3].dma_start(xt[:, k * G:(k + 1) * G],
                                          t["xt"][:, k * G:(k + 1) * G])
                nc.gpsimd.dma_start(wqk_sb[:], t["wqk"][:])

                bb = vps.tile([128, DL], f32, tag="va", bufs=4, name="bb")
                nc.tensor.matmul(bb[:], ones1[:], bv_sb[:],
                                 start=True, stop=True)
                nc.vector.tensor_copy(bv_bc[:], bb[:])
                for j in range(2):
                    bb2 = vps.tile([128, 384], f32, tag="va", bufs=4,
                                   name="bb2")
                    nc.tensor.matmul(bb2[:], ones1[:],
                                     bp_sb[:, j * 384:(j + 1) * 384],
                                     start=True, stop=True)
                    nc.vector.tensor_copy(bp_bc[:, j * 384:(j + 1) * 384],
                                          bb2[:])

                # token-tile groups of 4, k outer within the group
                for tg in range(4):
                    pas = []
                    for i in range(4):
                        pas.append(vps.tile([128, DL], f32, tag="va", bufs=4,
                                            name=f"pa{i}"))
                    for k in range(KCH):
                        for i in range(4):
                            tt = tg * 4 + i
                            lhsT = xt[:, k * G + tt * 128: k * G + tt * 128 + 128]
                            nc.tensor.matmul(pas[i][:], lhsT,
                                             wv_sb[:, k * DL: k * DL + DL],
                                             start=(k == 0), stop=(k == KCH - 1))
                    for i in range(4):
                        tt = tg * 4 + i
                        for hl in range(HL):
                            nc.vector.tensor_add(
                                vext[hl][tt][:, 0:D],
                                pas[i][:, hl * D:(hl + 1) * D],
                                bv_bc[:, hl * D:(hl + 1) * D])
                            nc.vector.tensor_copy(vext[hl][tt][:, D:D + 1],
                                                  onesK[:])

            # ---- QK phase -------------------------------------------------
            if True:
                for tblk in range(6):
                    wt = wqk_sb[:, tblk * 768:(tblk + 1) * 768]
                    is_k = tblk < 3
                    dest = kt_sb[tblk] if is_k else qt_sb[tblk - 3]
                    for n in range(4):
                        ps = qkps.tile([128, 512], f32, tag="qk", bufs=2)
                        tok0 = n * 512
                        for k in range(KCH):
                            nc.tensor.matmul(
                                ps[:], wt[:, k * 128:(k + 1) * 128],
                                xt[:, k * G + tok0: k * G + tok0 + 512],
                                start=(k == 0), stop=(k == KCH - 1))
                        nc.vector.tensor_scalar_add(
                            dest[:, tok0:tok0 + 512], ps[:],
                            bqk_sb[:, tblk:tblk + 1])

        # ---- attention + projection (xt freed) ---------------------------
        with tc.tile_pool(name="etps", bufs=2, space="PSUM") as et_ps, \
             tc.tile_pool(name="avps", bufs=2, space="PSUM") as av_ps, \
             tc.tile_pool(name="att_pool", bufs=4) as att_p, \
             tc.tile_pool(name="avs_pool", bufs=2) as avs_p, \
             tc.tile_pool(name="r_pool", bufs=2) as r_p, \
             tc.tile_pool(name="r1_pool", bufs=1) as r1_p, \
             tc.tile_pool(name="out_pool", bufs=2) as out_p, \
             tc.tile_pool(name="wp_pool", bufs=1) as wpp:
            wp_sb = wpp.tile([128, 3 * E], f16, tag="wp")
            nc.sync.dma_start(wp_sb[:], t["wp"][:])

            avs_tiles = {}

            # deferred work items (normalize / single proj qs-blocks) are
            # drained ONE PER QUAD inside the next pass so their PE matmuls
            # and DVE/DMA latencies hide behind ET/AV work instead of
            # running as a serial stall-prone block
            work_q = []
            # the previous pass's LAST quad of AV matmuls + reciprocal chain
            # are also deferred into the next pass's quad 0: they execute
            # while quad 0's exp runs, so quad 1's ET never waits on the
            # et-slot that exp must first release (pass-boundary bubble)
            carry = [None]

            def attn_head(qb, hl):
                avT0 = av_ps.tile([128, 512], f32, tag="avT0", name="avT0")
                avT1 = av_ps.tile([65, 512], f32, tag="avT1", name="avT1")
                chunks = _c_chunks(hl)
                # PE row-config switches (128-deep <-> 64-deep stationary)
                # cost ~96ns each, so batch same-depth ET matmuls: per quad
                # of 4 key-tiles, 4x 128-deep then 4x 64-deep. AV matmuls
                # (all 128-deep) for quad q are emitted during quad q+1's
                # ET block so they never wait on the exp activation.
                big = next(c for c in chunks if c[2] == 128)
                small = next(c for c in chunks if c[2] == 64)

                def emit_avs(kq, att_a, att_b):
                    for j in range(4):
                        kc = kq * 4 + j
                        att = att_a if j < 2 else att_b
                        sl = att[:, (j % 2) * 512:(j % 2 + 1) * 512]
                        vt = vext[hl][kc]
                        nc.tensor.matmul(avT0[:], vt[:, 0:128], sl,
                                         start=(kc == 0), stop=(kc == 15))
                        nc.tensor.matmul(avT1[:], vt[:, 128:193], sl,
                                         start=(kc == 0), stop=(kc == 15))

                pend_av = None
                for kq in range(4):
                    et_a = et_ps.tile([128, 1024], f32, tag="et", name="et_a")
                    et_b = et_ps.tile([128, 1024], f32, tag="et", name="et_b")
                    slots = [(et_a, 0), (et_a, 1), (et_b, 0), (et_b, 1)]
                    for depth_chunk, is_first in ((big, True), (small, False)):
                        ti, off, ln = depth_chunk
                        for j, (tile_, half) in enumerate(slots):
                            kc = kq * 4 + j
                            nc.tensor.matmul(
                                tile_[:, half * 512:(half + 1) * 512],
                                kt_sb[ti][off:off + ln, kc * 128:(kc + 1) * 128],
                                qt_sb[ti][off:off + ln, qb * 512:(qb + 1) * 512],
                                start=is_first, stop=not is_first)
                    att_a = att_p.tile([128, 1024], f16, tag="att", name="att_a")
                    nc.scalar.activation(att_a[:], et_a[:],
                                         mybir.ActivationFunctionType.Exp,
                                         scale=SCALE)
                    att_b = att_p.tile([128, 1024], f16, tag="att", name="att_b")
                    nc.scalar.activation(att_b[:], et_b[:],
                                         mybir.ActivationFunctionType.Exp,
                                         scale=SCALE)
                    if kq == 0 and carry[0] is not None:
                        carry[0]()
                        carry[0] = None
                    if pend_av is not None:
                        emit_avs(*pend_av)
                    pend_av = (kq, att_a, att_b)
                    # drain deferred work; at quad 0 only proj items (a
                    # normalize's bc matmul would stall on the previous
                    # pass's reciprocal chain, still in flight on Vector)
                    if work_q and (kq >= 1 or work_q[0][0] == 'proj'):
                        work_q.pop(0)[1]()

                def flush(qb=qb, hl=hl, avT0=avT0, avT1=avT1,
                          pend=pend_av, emit=emit_avs):
                    emit(*pend)
                    # reciprocal chain: one partition-shifted DVE copy (PSUM
                    # row 64 -> SBUF row 0), then fast reciprocal; f16 copy
                    # keeps the bc broadcast matmul on the 1-cycle f16 path
                    r0 = r1_p.tile([1, 512], f32, tag="r0", name="r0")
                    nc.vector.tensor_copy(r0[0:1, :], avT1[64:65, :])
                    rr32 = r_p.tile([1, 512], f32, tag="rr32", name="rr32")
                    nc.vector.reciprocal_approx_fast(rr32[:], r0[:])
                    rr = r_p.tile([1, 512], f16, tag="rr", name="rr")
                    nc.vector.tensor_copy(rr[:], rr32[:])
                    work_q.append(
                        ('norm',
                         lambda: normalize(qb, hl, avT0, avT1, rr)))
                    if hl == HL - 1:
                        for qs in range(4):
                            work_q.append(
                                ('proj',
                                 lambda qb=qb, qs=qs: proj_qs(qb, qs)))

                carry[0] = flush

            def normalize(qb, hl, avT0, avT1, rr):
                bc = et_ps.tile([128, 512], f32, tag="et", name="bc")
                nc.tensor.matmul(bc[:], ones1[:], rr[:], start=True, stop=True)
                bc_sb = r1_p.tile([128, 512], f32, tag="bcsb", name="bcsb")
                nc.vector.tensor_copy(bc_sb[:], bc[:])
                # pack avs into 3 full-128-partition tiles (local-d linear)
                # via partition-shifted DVE writes -> proj runs 3 contraction
                # chunks, all in the 128-row PE config
                if hl == 0:
                    P0 = avs_p.tile([128, 512], f16, tag="avsP0", name="P0")
                    P1 = avs_p.tile([128, 512], f16, tag="avsP1", name="P1")
                    avs_tiles[(qb, 0)] = P0
                    avs_tiles[(qb, 1)] = P1
                    nc.vector.tensor_mul(P0[:], avT0[0:128, :], bc_sb[0:128, :])
                    nc.vector.tensor_mul(P1[0:64, :], avT1[0:64, :],
                                         bc_sb[0:64, :])
                else:
                    P1 = avs_tiles[(qb, 1)]
                    P2 = avs_p.tile([128, 512], f16, tag="avsP2", name="P2")
                    avs_tiles[(qb, 2)] = P2
                    nc.vector.tensor_mul(P1[64:128, :], avT0[0:64, :],
                                         bc_sb[0:64, :])
                    nc.vector.tensor_mul(P2[0:64, :], avT0[64:128, :],
                                         bc_sb[64:128, :])
                    nc.vector.tensor_mul(P2[64:128, :], avT1[0:64, :],
                                         bc_sb[0:64, :])

            def proj_qs(qb, qs):
                p = et_ps.tile([128, 1024], f32, tag="et", name="p")
                p0, p1 = p[:, 0:384], p[:, 512:896]
                for cc in range(3):
                    lhsT = avs_tiles[(qb, cc)][:, qs * 128:(qs + 1) * 128]
                    nc.tensor.matmul(p0, lhsT,
                                     wp_sb[:, cc * 768: cc * 768 + 384],
                                     start=(cc == 0), stop=(cc == 2))
                    nc.tensor.matmul(p1, lhsT,
                                     wp_sb[:, cc * 768 + 384: cc * 768 + 768],
                                     start=(cc == 0), stop=(cc == 2))
                osb = out_p.tile([128, E], f16, tag="osb", name="osb")
                nc.vector.tensor_add(osb[:, 0:384], p0, bp_bc[:, 0:384])
                nc.vector.tensor_add(osb[:, 384:768], p1, bp_bc[:, 384:768])
                row = qb * 512 + qs * 128
                nc.sync.dma_start(t["y"][row:row + 128, :], osb[:])

            for qb in range(4):
                for hl in range(HL):
                    attn_head(qb, hl)
            carry[0]()
            carry[0] = None
            while work_q:
                work_q.pop(0)[1]()


_CACHED_NC = None


def _get_nc():
    global _CACHED_NC
    if _CACHED_NC is None:
        nc = bacc.Bacc("TRN2", target_bir_lowering=False, debug=False,
                       num_devices=N_CORES)
        t = {
            "xt": nc.dram_tensor("xt", (128, KCH * G), f16, kind="ExternalInput").ap(),
            "wqk": nc.dram_tensor("wqk", (128, 6 * 768), f16, kind="ExternalInput").ap(),
            "wv": nc.dram_tensor("wv", (128, KCH * DL), f16, kind="ExternalInput").ap(),
            "wp": nc.dram_tensor("wp", (128, 3 * E), f16, kind="ExternalInput").ap(),
            "bqk": nc.dram_tensor("bqk", (128, 6), f32, kind="ExternalInput").ap(),
            "bv": nc.dram_tensor("bv", (1, DL), f16, kind="ExternalInput").ap(),
            "bp": nc.dram_tensor("bp", (1, E), f16, kind="ExternalInput").ap(),
            "onesk": nc.dram_tensor("onesk", (128, 1), f16, kind="ExternalInput").ap(),
            "y": nc.dram_tensor("y", (G, E), f16, kind="ExternalOutput").ap(),
        }
        _emit(nc, t)
        nc.compile()
        _CACHED_NC = nc
    return _CACHED_NC


def _pack_contraction(w, rows=128):
    """(R, C) -> (rows, R//rows * C): contraction chunks on partitions."""
    r, c = w.shape
    n = r // rows
    return np.ascontiguousarray(
        w.reshape(n, rows, c).transpose(1, 0, 2).reshape(rows, n * c))


def make_in_maps(x, W_qkv, b_qkv, W_proj, b_proj):
    x = np.asarray(x, dtype=np.float32)
    W_qkv = np.asarray(W_qkv, dtype=np.float32)
    b_qkv = np.asarray(b_qkv, dtype=np.float32)
    W_proj = np.asarray(W_proj, dtype=np.float32)
    b_proj = np.asarray(b_proj, dtype=np.float32)

    # qkv column factorization: col = (h, d, {q,k,v}) with qkv fastest
    Wf = W_qkv.reshape(E, H, D, 3)
    bf = b_qkv.reshape(H, D, 3)

    hg_shared = []
    for hg in range(2):
        Wq = Wf[:, 2 * hg:2 * hg + 2, :, 0].reshape(E, DL)
        Wk = Wf[:, 2 * hg:2 * hg + 2, :, 1].reshape(E, DL)
        Wv = Wf[:, 2 * hg:2 * hg + 2, :, 2].reshape(E, DL)
        bq = bf[2 * hg:2 * hg + 2, :, 0].reshape(DL)
        bk = bf[2 * hg:2 * hg + 2, :, 1].reshape(DL)
        bv = bf[2 * hg:2 * hg + 2, :, 2].reshape(DL)

        blocks = [_pack_contraction(np.ascontiguousarray(
            Wk[:, i * 128:(i + 1) * 128])) for i in range(3)]
        blocks += [_pack_contraction(np.ascontiguousarray(
            Wq[:, i * 128:(i + 1) * 128])) for i in range(3)]
        wqk = np.concatenate(blocks, axis=1)  # (128, 6*768)
        bqk = np.stack([bk[i * 128:(i + 1) * 128] for i in range(3)]
                       + [bq[i * 128:(i + 1) * 128] for i in range(3)],
                       axis=1)  # (128, 6)

        wv_packed = _pack_contraction(np.ascontiguousarray(Wv))  # (128, 6*384)

        # W_proj rows for this head-group, packed 3 chunks of 128 rows
        # (matches the packed avs layout: local-d linear)
        Wp_hg = W_proj[hg * DL:(hg + 1) * DL]  # (384, 768)
        wp = _pack_contraction(np.ascontiguousarray(Wp_hg))  # (128, 3*768)

        bp = b_proj if hg == 0 else np.zeros_like(b_proj)
        hg_shared.append({
            "wqk": wqk.astype(np.float16),
            "wv": wv_packed.astype(np.float16),
            "wp": wp.astype(np.float16),
            "bqk": bqk,
            "bv": bv.reshape(1, DL).astype(np.float16),
            "bp": bp.reshape(1, E).astype(np.float16),
            "onesk": np.ones((128, 1), dtype=np.float16),
        })

    in_maps = []
    for c in range(N_CORES):
        b, hg = divmod(c, 2)
        xt = _pack_contraction(np.ascontiguousarray(x[b].T))  # (128, 6*2048)
        in_maps.append({"xt": xt.astype(np.float16), **hg_shared[hg]})
    return in_maps


def kernel(**inputs):
    nc = _get_nc()
    in_maps = make_in_maps(inputs["x"], inputs["W_qkv"], inputs["b_qkv"],
                           inputs["W_proj"], inputs["b_proj"])
    res = run_bass_kernel_spmd(nc, in_maps, core_ids=list(range(N_CORES)))
    out = np.empty((B, G, E), dtype=np.float32)
    for b in range(B):
        out[b] = (res.results[2 * b]["y"].astype(np.float32)
                  + res.results[2 * b + 1]["y"].astype(np.float32))
    return out
